# revision 9
# baseline (speedup 1.0000x reference)
"""CapsNet dynamic-routing layer on 8 Trainium2 NeuronCores (Bass/Tile).

reference math (per batch element b):
  u_hat[b,i,o,j] = sum_d W[i,o,j,d] * u[b,i,d]        (never materialized)
  bl = 0; for r in 0..2:
    c = softmax_o(bl); s[b,o,j] = sum_i c*u_hat; v = squash(s)
    if r < 2: bl += sum_j u_hat*v
  return v  [B, 10, 16]

Distribution: pure data parallel, batch 512 -> 64 per core x 8 cores;
weights replicated. Per-core sizes: b=64, i=1152=9*128, o=10, j=16, d=8.

Per-core kernel structure:
 - pass A (s-step): cu[i,(o,d,b)] = c*u built on DVE (bf16), then per-o
   PE matmuls contract K=i (9 tiles x 8 d, PSUM-accumulated) against
   W-slices [i128, j16].  iter 0 uses c=0.1 uniform -> rhs = u directly.
 - squash entirely in j-partitioned form: sum_j s^2 via ones-matmul,
   per-(o,b) scalars on a 1-partition strip, replicated back to 16
   partitions via a K=1 ones-matmul; sqrt as exp(0.5*ln) so only the
   natural_log_exp ACT table set is used.
 - pass B (agreement): g[b,(o),(d,i)] = sum_j W*v via PE (K=j=16,
   row-tiled over 4 row groups, two o's packed per PSUM tile in
   partition halves), evacuated by ACT to bf16, multiplied by u on DVE,
   d-reduced by contiguous tree adds, accumulated into f32 b-logits.
 - softmax over o: the o dim is split as (h = o//5) across partition
   halves and (o5 = o%5) along free; half-sum exchange via identity
   matmuls; reciprocal via fast NR approximation; c transposed to the
   i-partitioned layout with 90 DMA-transposes ([64,128] bf16 tiles).
"""
import sys
import os

sys.path.insert(0, "/opt/trn_rl_repo")

import numpy as np
import ml_dtypes
from contextlib import ExitStack

from concourse import bacc, mybir
from concourse.tile import TileContext
from concourse.bass_utils import run_bass_kernel_spmd

BF16 = mybir.dt.bfloat16
F32 = mybir.dt.float32
AX = mybir.AxisListType
ALU = mybir.AluOpType
ACTF = mybir.ActivationFunctionType
bfnp = ml_dtypes.bfloat16

B = 64
I = 1152
T = 9
O = 10
J = 16
D = 8
EPS = 1e-06
N_CORES = 8
KFLAT = D * I          # 9216 (d-major flat)
NCH = KFLAT // 512     # 18

# Pass-B o ordering: position k -> row group k%4, slot k//4.
# Pairs (p, p+5) share a PSUM tile (partition halves), matching the
# softmax layout h=o//5.  Per-group o sets are arithmetic (stride 2).
OMEGA = [0, 5, 1, 6, 2, 7, 3, 8, 4, 9]
RG = {o: k % 4 for k, o in enumerate(OMEGA)}
SLOT = {o: k // 4 for k, o in enumerate(OMEGA)}

_cache = {}


def build_nc():
    nc = bacc.Bacc()
    ws_d = nc.dram_tensor("ws", [128, T, D, O, J], BF16, kind="ExternalInput")
    wb_d = nc.dram_tensor("wb", [128, 3 * KFLAT], BF16, kind="ExternalInput")
    ui_d = nc.dram_tensor("ui", [128, T, D, B], BF16, kind="ExternalInput")
    ur_d = nc.dram_tensor("ur", [128, KFLAT], BF16, kind="ExternalInput")
    cid2_d = nc.dram_tensor("cid2", [128, 64], F32, kind="ExternalInput")
    cones16_d = nc.dram_tensor("cones16", [16, 1], F32, kind="ExternalInput")
    cones1_d = nc.dram_tensor("cones1", [1, 16], F32, kind="ExternalInput")
    cid16_d = nc.dram_tensor("cid16", [16, 16], BF16, kind="ExternalInput")
    vout_d = nc.dram_tensor("vout", [B, O, J], F32, kind="ExternalOutput")

    with TileContext(nc) as tc, ExitStack() as ctx:
        static = ctx.enter_context(tc.tile_pool(name="static", bufs=1))
        work = ctx.enter_context(tc.tile_pool(name="work", bufs=1))
        cupool = ctx.enter_context(tc.tile_pool(name="cup", bufs=2))
        psA = ctx.enter_context(tc.tile_pool(name="psA", bufs=2, space="PSUM"))
        psB = ctx.enter_context(tc.tile_pool(name="psB", bufs=4, space="PSUM"))
        psC = ctx.enter_context(tc.tile_pool(name="psC", bufs=2, space="PSUM"))

        ws = static.tile([128, T, D, O, J], BF16, name="ws")
        wb = static.tile([128, 3 * KFLAT], BF16, name="wb")
        ui = static.tile([128, T, D, B], BF16, name="ui")
        ur = static.tile([128, KFLAT], BF16, name="ur")
        cid2 = static.tile([128, 64], F32, name="cid2")
        cones16 = static.tile([16, 1], F32, name="cones16")
        cones1 = static.tile([1, 16], F32, name="cones1")
        cid16 = static.tile([16, 16], BF16, name="cid16")
        nc.sync.dma_start(out=ws, in_=ws_d[:, :, :, :, :])
        nc.sync.dma_start(out=wb, in_=wb_d[:, :])
        nc.sync.dma_start(out=ui, in_=ui_d[:, :, :, :])
        nc.sync.dma_start(out=ur, in_=ur_d[:, :])
        nc.sync.dma_start(out=cid2, in_=cid2_d[:, :])
        nc.sync.dma_start(out=cones16, in_=cones16_d[:, :])
        nc.sync.dma_start(out=cones1, in_=cones1_d[:, :])
        nc.sync.dma_start(out=cid16, in_=cid16_d[:, :])

        bl = work.tile([128, 5, I], F32, name="bl")
        c_t = work.tile([128, T, O, B], BF16, name="c_t")
        zh = work.tile([128, I], F32, name="zh")
        rz = work.tile([128, I], F32, name="rz")
        scratch = work.tile([128, KFLAT], BF16, name="scratch")
        ug = scratch
        e = scratch[:, 0 : 5 * I].rearrange("p (o i) -> p o i", o=5)
        s_sb = work.tile([16, O, B], F32, name="s_sb")
        s2 = work.tile([16, O * B], F32, name="s2")
        v_jb = work.tile([16, O, B], F32, name="v_jb")
        v_bf = work.tile([16, O, B], BF16, name="v_bf")
        v_st = work.tile([128, 3, B], BF16, name="v_st")
        t1p = work.tile([1, O * B], F32, name="t1p")
        t2p = work.tile([1, O * B], F32, name="t2p")
        den = work.tile([1, O * B], F32, name="den")
        rec = work.tile([1, O * B], F32, name="rec")
        v_t = work.tile([64, O, J], F32, name="v_t")
        eps1 = work.tile([1, 1], F32, name="eps1")
        nc.vector.memset(eps1, EPS)

        def m1_o(o, rhs_tile, scale):
            """s_o = sum_{t,d} Ws[:,t,d,o,:].T @ rhs[:,t,d,:] -> s_sb[:, o, :]."""
            ps = psA.tile([16, B], F32, name="m1ps", tag="m1ps")
            for t in range(T):
                for d in range(D):
                    nc.tensor.matmul(
                        ps,
                        ws[:, t, d, o, :],
                        rhs_tile[:, t, d, :],
                        start=(t == 0 and d == 0),
                        stop=(t == T - 1 and d == D - 1),
                    )
            nc.scalar.mul(s_sb[:, o, :], ps, scale)

        def squash():
            """v_jb = squash(s_sb) along j (the partition dim, via matmuls)."""
            sf = s_sb.rearrange("p o b -> p (o b)")
            nc.vector.tensor_tensor(s2, sf, sf, op=ALU.mult)
            for off, n in ((0, 512), (512, 128)):
                sq = psC.tile([1, 512], F32, name="sqps", tag="miscps")[:, :n]
                nc.tensor.matmul(sq, cones16, s2[:, off : off + n],
                                 start=True, stop=True)
                sl = slice(off, off + n)
                nc.scalar.activation(t1p[:, sl], sq, ACTF.Ln, bias=eps1)
                nc.scalar.activation(t2p[:, sl], t1p[:, sl], ACTF.Exp, scale=0.5)
                nc.vector.tensor_scalar_add(den[:, sl], sq, 1.0)
                nc.vector.tensor_tensor(den[:, sl], den[:, sl], t2p[:, sl],
                                        op=ALU.mult)
                nc.vector.reciprocal_approx_accurate(rec[:, sl], den[:, sl], t1p[:, sl])
                nc.vector.tensor_tensor(den[:, sl], sq, rec[:, sl], op=ALU.mult)
                mrep = psC.tile([16, 512], F32, name="mrps", tag="miscps")[:, :n]
                nc.tensor.matmul(mrep, cones1, den[:, sl], start=True, stop=True)
                vf = v_jb.rearrange("p o b -> p (o b)")
                nc.vector.tensor_tensor(vf[:, sl], sf[:, sl], mrep, op=ALU.mult)

        def v_prep():
            """v_jb -> v_st[32*rg+jj, slot, :] bf16 (partition moves via PE)."""
            nc.vector.tensor_copy(v_bf[:, :, :], v_jb[:, :, :])
            vmov = psC.tile([128, 3 * B], F32, name="vmov", tag="miscps")
            for o in range(O):
                rg, sl = RG[o], SLOT[o]
                nc.tensor.matmul(
                    vmov[32 * rg : 32 * rg + 16, sl * B : (sl + 1) * B],
                    cid16,
                    v_bf[:, o, :],
                    start=True,
                    stop=True,
                    tile_position=(0, 32 * rg),
                )
            for rg in range(4):
                nsl = 3 if rg < 2 else 2
                nc.scalar.copy(
                    v_st[32 * rg : 32 * rg + 16, 0:nsl, :],
                    vmov[32 * rg : 32 * rg + 16, 0 : nsl * B].rearrange(
                        "p (s b) -> p s b", s=nsl
                    ),
                )

        def m2_b2(it):
            """bl (+)= sum_j u_hat * v, via g = W.T@v then u*g and d-tree."""
            for p in range(5):
                for n in range(NCH):
                    ps = psB.tile([128, 512], F32, name="m2ps", tag="m2ps")
                    for half, o in ((0, p), (1, p + 5)):
                        rg, sl = RG[o], SLOT[o]
                        nc.tensor.matmul(
                            ps[64 * half : 64 * half + 64, :],
                            v_st[32 * rg : 32 * rg + 16, sl, :],
                            wb[32 * rg : 32 * rg + 16,
                               sl * KFLAT + 512 * n : sl * KFLAT + 512 * (n + 1)],
                            start=True,
                            stop=True,
                            tile_position=(32 * rg, 64 * half),
                        )
                    nc.scalar.copy(ug[:, 512 * n : 512 * (n + 1)], ps)
                for n in range(NCH):
                    sl = slice(512 * n, 512 * (n + 1))
                    nc.vector.tensor_tensor(ug[:, sl], ug[:, sl], ur[:, sl],
                                            op=ALU.mult)
                h, q = KFLAT // 2, KFLAT // 4
                nc.vector.tensor_tensor(ug[:, 0:h], ug[:, 0:h], ug[:, h:KFLAT],
                                        op=ALU.add)
                nc.vector.tensor_tensor(ug[:, h : h + q], ug[:, 0:q], ug[:, q:h],
                                        op=ALU.add)
                l3 = ug[:, 6912:9216].bitcast(F32)
                nc.vector.tensor_tensor(l3, ug[:, h : h + I],
                                        ug[:, h + I : h + 2 * I], op=ALU.add)
                if it == 0:
                    nc.vector.tensor_copy(bl[:, p, :], l3)
                else:
                    nc.vector.tensor_tensor(bl[:, p, :], bl[:, p, :], l3, op=ALU.add)

        def softmax():
            """e := c = softmax_o(bl); then c -> c_t (i-partitioned)."""
            nc.scalar.activation(e[:, :, :], bl[:, :, :], ACTF.Exp)
            nc.vector.tensor_reduce(zh, e.transpose([0, 2, 1]), axis=AX.X,
                                    op=ALU.add)
            for n in range(3):
                sl = slice(384 * n, 384 * (n + 1))
                zp = psC.tile([128, 384], F32, name="zswap", tag="miscps")
                nc.tensor.matmul(zp[0:64, :], cid2[64:128, :], zh[64:128, sl],
                                 start=True, stop=True, tile_position=(64, 0))
                nc.tensor.matmul(zp[64:128, :], cid2[0:64, :], zh[0:64, sl],
                                 start=True, stop=True, tile_position=(0, 64))
                nc.vector.tensor_tensor(zh[:, sl], zh[:, sl], zp, op=ALU.add)
            nc.vector.reciprocal_approx_fast(rz, zh)
            for o5 in range(5):
                nc.vector.tensor_tensor(e[:, o5, :], e[:, o5, :], rz, op=ALU.mult)
            for o5 in range(5):
                for t in range(T):
                    for hh in range(2):
                        nc.sync.dma_start_transpose(
                            out=c_t[:, t, o5 + 5 * hh, :],
                            in_=e[64 * hh : 64 * hh + 64, o5,
                                  128 * t : 128 * (t + 1)],
                        )

        # ========================= flow =========================
        for it in range(3):
            if it == 0:
                for o in range(O):
                    m1_o(o, ui, 0.1)
            else:
                for o in range(O):
                    cu = cupool.tile([128, T, D, B], BF16, name="cu", tag="cu")
                    for d in range(D):
                        nc.vector.tensor_tensor(
                            cu[:, :, d, :], c_t[:, :, o, :], ui[:, :, d, :],
                            op=ALU.mult,
                        )
                    m1_o(o, cu, 1.0)
            squash()
            if it < 2:
                v_prep()
                m2_b2(it)
                softmax()

        # final output: transpose v_jb [16,(o,b)] -> v_t [64,(o,j)] -> DRAM
        for o in range(O):
            tp = psC.tile([64, J], F32, name="vtp", tag="miscps")
            nc.tensor.transpose(tp, v_jb[:, o, :], cid2[0:16, 0:16])
            nc.scalar.copy(v_t[:, o, :], tp)
        nc.sync.dma_start(out=vout_d[:, :, :], in_=v_t)

    nc.finalize()
    return nc


def _host_prep(u, weights):
    """Build per-core input maps. u [512,1152,8] f32, weights [1152,10,16,8]."""
    W = np.asarray(weights, dtype=np.float32)
    u = np.asarray(u, dtype=np.float32)
    ws = np.ascontiguousarray(
        W.reshape(T, 128, O, J, D).transpose(1, 0, 4, 2, 3)
    ).astype(bfnp)  # [128, T, D, O, J]
    wt = W.transpose(1, 2, 3, 0)  # [o, j, d, i]
    wb = np.zeros((128, 3 * KFLAT), dtype=bfnp)
    for k, o in enumerate(OMEGA):
        rg, sl = k % 4, k // 4
        wb[32 * rg : 32 * rg + 16, sl * KFLAT : (sl + 1) * KFLAT] = (
            wt[o].reshape(J, KFLAT).astype(bfnp)
        )
    cid2 = np.concatenate([np.eye(64, dtype=np.float32)] * 2, axis=0)
    cones16 = np.ones((16, 1), dtype=np.float32)
    cones1 = np.ones((1, 16), dtype=np.float32)
    cid16 = np.eye(16, dtype=np.float32).astype(bfnp)

    in_maps = []
    for c in range(N_CORES):
        uc = u[c * B : (c + 1) * B]  # [64, 1152, 8]
        ui = np.ascontiguousarray(
            uc.reshape(B, T, 128, D).transpose(2, 1, 3, 0)
        ).astype(bfnp)  # [128, T, D, B]
        urh = np.ascontiguousarray(uc.transpose(0, 2, 1)).reshape(B, KFLAT)
        ur = np.concatenate([urh, urh], axis=0).astype(bfnp)  # [128, KFLAT]
        in_maps.append({
            "ws": ws, "wb": wb, "ui": ui, "ur": ur,
            "cid2": cid2, "cones16": cones16, "cones1": cones1, "cid16": cid16,
        })
    return in_maps


def kernel(u, weights):
    if "nc" not in _cache:
        _cache["nc"] = build_nc()
    nc = _cache["nc"]
    in_maps = _host_prep(u, weights)
    res = run_bass_kernel_spmd(nc, in_maps, core_ids=list(range(N_CORES)))
    out = np.concatenate([res.results[c]["vout"] for c in range(N_CORES)], axis=0)
    return out.astype(np.float32)


if __name__ == "__main__":
    rng = np.random.default_rng(0)
    u = rng.standard_normal((512, 1152, 8), dtype=np.float32)
    w = (rng.standard_normal((1152, 10, 16, 8)) * 0.1).astype(np.float32)
    v = kernel(u, w)
    print("out", v.shape, v.dtype, np.abs(v).max())


# revision 23
# speedup vs baseline: 7855.7835x; 7855.7835x over previous
"""CapsNet dynamic-routing layer on 8 Trainium2 NeuronCores (Bass/Tile).

reference math (per batch element b):
  u_hat[b,i,o,j] = sum_d W[i,o,j,d] * u[b,i,d]        (never materialized)
  bl = 0; for r in 0..2:
    c = softmax_o(bl); s[b,o,j] = sum_i c*u_hat; v = squash(s)
    if r < 2: bl += sum_j u_hat*v
  return v  [B, 10, 16]

Distribution: pure data parallel, batch 512 -> 64 per core x 8 cores;
weights replicated.  Per-core: b=64, i=1152=9*128, o=10, j=16, d=8.

Key layout trick: o is mapped to PE column/row strips as g=o%4 (strip)
and sl=o//4 (slot), consistently across the s-matmuls (col-tiled),
squash (strip-local), the agreement matmuls (row-tiled), and the
output transposes - so no partition-moving shuffles are ever needed.
The softmax splits o as h=o//5 across partition halves (paired with
the agreement-pass PSUM packing) and o5=o%5 along free.
"""
import sys

sys.path.insert(0, "/opt/trn_rl_repo")

import numpy as np
import ml_dtypes
from contextlib import ExitStack

from concourse import bacc, mybir, hw_specs
from concourse.tile import TileContext
from concourse.bass_utils import run_bass_kernel_spmd

BF16 = mybir.dt.bfloat16
F32 = mybir.dt.float32
AX = mybir.AxisListType
ALU = mybir.AluOpType
ACTF = mybir.ActivationFunctionType
bfnp = ml_dtypes.bfloat16

B = 64
I = 1152
T = 9
O = 10
J = 16
D = 8
EPS = 1e-06
N_CORES = 8
KFLAT = D * I          # 9216 (d-major flat)
NCH = KFLAT // 512     # 18

_cache = {}

# Route every activation through the one table set that has exp+ln+copy,
# so the ACT engine never reloads tables mid-kernel.  Entry order (and
# hence act_func_set_id indices) is preserved.
_KEEP_SET = "natural_log_exp_and_others"


def _patched_tables(arch):
    full = {k: set(v) for k, v in hw_specs.get_activation_tables(arch).items()}
    keep = full[_KEEP_SET]
    return {k: (v if k == _KEEP_SET else v - keep) for k, v in full.items()}


import os
if os.environ.get('ACT_PATCH', '1') == '1':
    bacc.get_activation_tables = _patched_tables


def build_nc():
    nc = bacc.Bacc()
    ws_d = nc.dram_tensor("ws", [128, T, D, O, J], BF16, kind="ExternalInput")
    wb_d = nc.dram_tensor("wb", [128, 3 * KFLAT], BF16, kind="ExternalInput")
    ui_d = nc.dram_tensor("ui", [128, T, D, B], BF16, kind="ExternalInput")
    ur_d = nc.dram_tensor("ur", [128, KFLAT], BF16, kind="ExternalInput")
    cid2_d = nc.dram_tensor("cid2", [128, 64], F32, kind="ExternalInput")
    cid16s_d = nc.dram_tensor("cid16s", [128, 16], F32, kind="ExternalInput")
    cones128_d = nc.dram_tensor("cones128", [128, 1], F32, kind="ExternalInput")
    cones1_d = nc.dram_tensor("cones1", [1, 16], F32, kind="ExternalInput")
    vout_d = nc.dram_tensor("vout", [B, O, J], F32, kind="ExternalOutput")

    with TileContext(nc) as tc, ExitStack() as ctx:
        static = ctx.enter_context(tc.tile_pool(name="static", bufs=1))
        work = ctx.enter_context(tc.tile_pool(name="work", bufs=1))
        cupool = ctx.enter_context(tc.tile_pool(name="cup", bufs=2))
        psA = ctx.enter_context(tc.tile_pool(name="psA", bufs=2, space="PSUM"))
        psB = ctx.enter_context(tc.tile_pool(name="psB", bufs=2, space="PSUM"))
        psC = ctx.enter_context(tc.tile_pool(name="psC", bufs=2, space="PSUM"))
        psD = ctx.enter_context(tc.tile_pool(name="psD", bufs=2, space="PSUM"))

        ws = static.tile([128, T, D, O, J], BF16, name="ws")
        wb = static.tile([128, 3 * KFLAT], BF16, name="wb")
        ui = static.tile([128, T, D, B], BF16, name="ui")
        ur = static.tile([128, KFLAT], BF16, name="ur")
        cid2 = static.tile([128, 64], F32, name="cid2")
        cid16s = static.tile([128, 16], F32, name="cid16s")
        cones128 = static.tile([128, 1], F32, name="cones128")
        cones1 = static.tile([1, 16], F32, name="cones1")
        nc.sync.dma_start(out=ws, in_=ws_d[:, :, :, :, :])
        nc.sync.dma_start(out=wb, in_=wb_d[:, :])
        nc.sync.dma_start(out=ui, in_=ui_d[:, :, :, :])
        nc.sync.dma_start(out=ur, in_=ur_d[:, :])
        nc.sync.dma_start(out=cid2, in_=cid2_d[:, :])
        nc.sync.dma_start(out=cid16s, in_=cid16s_d[:, :])
        nc.sync.dma_start(out=cones128, in_=cones128_d[:, :])
        nc.sync.dma_start(out=cones1, in_=cones1_d[:, :])

        bl = work.tile([128, 5, I], F32, name="bl")
        c_t = work.tile([128, T, O, B], BF16, name="c_t")
        zh = work.tile([128, I], F32, name="zh")
        rz = work.tile([128, I], F32, name="rz")
        scratch = work.tile([128, KFLAT], BF16, name="scratch")
        ug = scratch
        e = scratch[:, 0 : 5 * I].rearrange("p (o i) -> p o i", o=5)
        s_sb = work.tile([128, 3, B], F32, name="s_sb")
        s2 = work.tile([128, 3, B], F32, name="s2")
        v_sb = work.tile([128, 3, B], F32, name="v_sb")
        v_st = work.tile([128, 3, B], BF16, name="v_st")
        sq_sb = work.tile([1, 4, 3, B], F32, name="sq_sb")
        t1p = work.tile([1, 768], F32, name="t1p")
        t2p = work.tile([1, 768], F32, name="t2p")
        den = work.tile([1, 768], F32, name="den")
        rec = work.tile([1, 768], F32, name="rec")
        v_t = work.tile([64, O, J], F32, name="v_t")
        eps1 = work.tile([1, 1], F32, name="eps1")
        nc.vector.memset(eps1, EPS)
        nc.vector.memset(s_sb.rearrange("p s b -> p (s b)"), 0.0)
        nc.vector.memset(sq_sb.rearrange("p g s b -> p (g s b)"), 0.0)

        PAIRS = [(0, 1), (2, 3), (4, 5), (6, 7), (8, 9)]

        def m1_pair(pair, rhs_of, scale):
            """col-tiled s matmuls for an o-pair -> s_sb strips."""
            ps = psA.tile([128, B], F32, name="m1ps", tag="m1ps")
            for t in range(T):
                for d in range(D):
                    for o in pair:
                        g = o % 4
                        nc.tensor.matmul(
                            ps[32 * g : 32 * g + 16, :],
                            ws[:, t, d, o, :],
                            rhs_of(o)[:, t, d, :],
                            start=(t == 0 and d == 0),
                            stop=(t == T - 1 and d == D - 1),
                            tile_position=(0, 32 * g),
                        )
            for o in pair:
                g, slot = o % 4, o // 4
                nc.scalar.mul(s_sb[32 * g : 32 * g + 16, slot, :],
                              ps[32 * g : 32 * g + 16, :], scale)

        def squash():
            """v_sb = squash(s_sb) with j on partitions (strip-local)."""
            sf = s_sb.rearrange("p s b -> p (s b)")
            s2f = s2.rearrange("p s b -> p (s b)")
            nc.vector.tensor_tensor(s2f, sf, sf, op=ALU.mult)
            for g in range(4):
                nsl = 3 if g < 2 else 2
                sqg = psD.tile([1, 3 * B], F32, name="sqg", tag="sqps")
                nc.tensor.matmul(
                    sqg[:, 0 : nsl * B],
                    cones128[32 * g : 32 * g + 16, :],
                    s2[32 * g : 32 * g + 16, 0:nsl, :],
                    start=True, stop=True,
                    tile_position=(32 * g, 0),
                )
                # scatter group's o-slices (o = g + 4*sl) into sq_sb
                nc.vector.tensor_copy(
                    sq_sb[:, g, 0:nsl, :],
                    sqg[:, 0 : nsl * B].rearrange("p (s b) -> p s b", s=nsl),
                )
            # o-major view of sq_sb: o = g + 4*sl  ->  dims (sl, g, b)
            sqv = sq_sb.transpose([0, 2, 1, 3])
            def _v(ap):
                return ap.rearrange("p (s g b) -> p s g b", s=3, g=4)
            nc.scalar.activation(_v(t1p), sqv, ACTF.Ln, bias=eps1)
            nc.scalar.activation(t2p, t1p, ACTF.Exp, scale=0.5)
            nc.vector.tensor_scalar_add(_v(den), sqv, 1.0)
            nc.vector.tensor_tensor(den, den, t2p, op=ALU.mult)
            nc.vector.reciprocal_approx_accurate(rec, den, t1p)
            nc.vector.tensor_tensor(_v(den), sqv, _v(rec), op=ALU.mult)
            mrep = psC.tile([128, 3 * B], F32, name="mrep", tag="miscps")
            nc.vector.memset(mrep, 0.0)
            for o in range(O):
                g, sl = o % 4, o // 4
                nc.tensor.matmul(
                    mrep[32 * g : 32 * g + 16, 64 * sl : 64 * (sl + 1)],
                    cones1,
                    den[:, 64 * o : 64 * (o + 1)],
                    start=True, stop=True,
                    tile_position=(0, 32 * g),
                )
            vf = v_sb.rearrange("p s b -> p (s b)")
            nc.vector.tensor_tensor(vf, sf, mrep, op=ALU.mult)

        def m2_b2(it):
            """bl (+)= sum_j u_hat * v   (g = W.T@v row-tiled; u*g; d-tree)."""
            nc.vector.tensor_copy(v_st.rearrange("p s b -> p (s b)"),
                                  v_sb.rearrange("p s b -> p (s b)"))
            for p in range(5):
                for n in range(NCH // 2):
                    for nn in (n, n + 9):
                        ps = psB.tile([128, 512], F32, name="m2ps", tag="m2ps")
                        for half, o in ((0, p), (1, p + 5)):
                            g, sl = o % 4, o // 4
                            nc.tensor.matmul(
                                ps[64 * half : 64 * half + 64, :],
                                v_st[32 * g : 32 * g + 16, sl, :],
                                wb[32 * g : 32 * g + 16,
                                   sl * KFLAT + 512 * nn : sl * KFLAT + 512 * (nn + 1)],
                                start=True, stop=True,
                                tile_position=(32 * g, 64 * half),
                            )
                        nc.scalar.copy(ug[:, 512 * nn : 512 * (nn + 1)], ps)
                    for nn in (n, n + 9):
                        sl = slice(512 * nn, 512 * (nn + 1))
                        nc.vector.tensor_tensor(ug[:, sl], ug[:, sl], ur[:, sl],
                                                op=ALU.mult)
                    sl = slice(512 * (n + 9), 512 * (n + 10))
                    nc.vector.tensor_tensor(
                        ug[:, sl], ug[:, 512 * n : 512 * (n + 1)],
                        ug[:, sl], op=ALU.add)
                h, q = KFLAT // 2, KFLAT // 4
                # l1 lives in [h:KFLAT); fold its halves into [h:h+q)
                nc.vector.tensor_tensor(ug[:, h : h + q], ug[:, h : h + q],
                                        ug[:, h + q : KFLAT], op=ALU.add)
                l3 = ug[:, h + q : h + q + 2 * I].bitcast(F32)
                nc.vector.tensor_tensor(l3, ug[:, h : h + I],
                                        ug[:, h + I : h + 2 * I], op=ALU.add)
                if it == 0:
                    nc.vector.tensor_copy(bl[:, p, :], l3)
                else:
                    nc.vector.tensor_tensor(bl[:, p, :], bl[:, p, :], l3,
                                            op=ALU.add)

        def softmax():
            """e := c = softmax_o(bl); c -> c_t (i-partitioned) via DMA-T."""
            nc.scalar.activation(e[:, :, :], bl[:, :, :], ACTF.Exp)
            nc.vector.tensor_tensor(zh, e[:, 0, :], e[:, 1, :], op=ALU.add)
            nc.vector.tensor_tensor(rz, e[:, 2, :], e[:, 3, :], op=ALU.add)
            nc.vector.tensor_tensor(zh, zh, e[:, 4, :], op=ALU.add)
            nc.vector.tensor_tensor(zh, zh, rz, op=ALU.add)
            for n in range(3):
                sl = slice(384 * n, 384 * (n + 1))
                zp = psC.tile([128, 384], F32, name="zswap", tag="miscps")
                nc.tensor.matmul(zp[0:64, :], cid2[64:128, :], zh[64:128, sl],
                                 start=True, stop=True, tile_position=(64, 0))
                nc.tensor.matmul(zp[64:128, :], cid2[0:64, :], zh[0:64, sl],
                                 start=True, stop=True, tile_position=(0, 64))
                nc.vector.tensor_tensor(zh[:, sl], zh[:, sl], zp, op=ALU.add)
            nc.vector.reciprocal_approx_fast(rz, zh)
            for o5 in range(5):
                nc.vector.tensor_tensor(e[:, o5, :], e[:, o5, :], rz,
                                        op=ALU.mult)
            for o in range(O):
                o5, hh = o % 5, o // 5
                for t in range(T):
                    nc.sync.dma_start_transpose(
                        out=c_t[:, t, o, :],
                        in_=e[64 * hh : 64 * hh + 64, o5,
                              128 * t : 128 * (t + 1)],
                    )

        # ========================= flow =========================
        import os as _os
        STAGE = int(_os.environ.get("FLOW_STAGE", "99"))
        for it in range(3):
            if it > 0 and STAGE < 4:
                break
            if it == 0:
                for pair in PAIRS:
                    m1_pair(pair, lambda o: ui, 0.1)
            else:
                for pair in PAIRS:
                    cus = {}
                    for o in pair:
                        cu = cupool.tile([128, T, D, B], BF16, name="cu",
                                         tag="cu")
                        nc.vector.tensor_tensor(
                            cu[:, :, :, :],
                            c_t[:, :, o, :].unsqueeze(2).broadcast_to(
                                [128, T, D, B]),
                            ui[:, :, :, :],
                            op=ALU.mult,
                        )
                        cus[o] = cu
                    m1_pair(pair, lambda o: cus[o], 1.0)
            if STAGE >= 1:
                squash()
            if it < 2 and STAGE >= 2:
                m2_b2(it)
                if STAGE >= 3:
                    softmax()

        if STAGE < 1:
            nc.vector.memset(v_sb.rearrange("p s b -> p (s b)"), 0.5)
        for o in range(O):
            g, sl = o % 4, o // 4
            tp = psC.tile([64, J], F32, name="vtp", tag="miscps")
            nc.tensor.transpose(tp, v_sb[32 * g : 32 * g + 16, sl, :],
                                cid16s[32 * g : 32 * g + 16, :],
                                tile_position=(32 * g, 0))
            nc.scalar.copy(v_t[:, o, :], tp)
        nc.sync.dma_start(out=vout_d[:, :, :], in_=v_t)

    nc.finalize()
    return nc


def _host_prep(u, weights):
    """Per-core input maps. u [512,1152,8] f32, weights [1152,10,16,8] f32."""
    W = np.asarray(weights, dtype=np.float32)
    u = np.asarray(u, dtype=np.float32)
    ws = np.ascontiguousarray(
        W.reshape(T, 128, O, J, D).transpose(1, 0, 4, 2, 3)
    ).astype(bfnp)  # [128, T, D, O, J]
    wt = W.transpose(1, 2, 3, 0)  # [o, j, d, i]
    wb = np.zeros((128, 3 * KFLAT), dtype=bfnp)
    for o in range(O):
        g, sl = o % 4, o // 4
        wb[32 * g : 32 * g + 16, sl * KFLAT : (sl + 1) * KFLAT] = (
            wt[o].reshape(J, KFLAT).astype(bfnp)
        )
    cid2 = np.concatenate([np.eye(64, dtype=np.float32)] * 2, axis=0)
    cid16s = np.zeros((128, 16), dtype=np.float32)
    for g in range(4):
        cid16s[32 * g : 32 * g + 16, :] = np.eye(16, dtype=np.float32)
    cones128 = np.ones((128, 1), dtype=np.float32)
    cones1 = np.ones((1, 16), dtype=np.float32)

    base = {
        "ws": ws, "wb": wb, "cid2": cid2, "cid16s": cid16s,
        "cones128": cones128, "cones1": cones1,
    }
    in_maps = []
    for c in range(N_CORES):
        uc = u[c * B : (c + 1) * B]  # [64, 1152, 8]
        ui = np.ascontiguousarray(
            uc.reshape(B, T, 128, D).transpose(2, 1, 3, 0)
        ).astype(bfnp)  # [128, T, D, B]
        urh = np.ascontiguousarray(uc.transpose(0, 2, 1)).reshape(B, KFLAT)
        ur = np.concatenate([urh, urh], axis=0).astype(bfnp)  # [128, KFLAT]
        in_maps.append({**base, "ui": ui, "ur": ur})
    return in_maps


def kernel(u, weights):
    if "nc" not in _cache:
        _cache["nc"] = build_nc()
    nc = _cache["nc"]
    in_maps = _host_prep(u, weights)
    res = run_bass_kernel_spmd(nc, in_maps, core_ids=list(range(N_CORES)))
    out = np.concatenate([res.results[c]["vout"] for c in range(N_CORES)], axis=0)
    return out.astype(np.float32)


if __name__ == "__main__":
    rng = np.random.default_rng(0)
    u = rng.standard_normal((512, 1152, 8), dtype=np.float32)
    w = (rng.standard_normal((1152, 10, 16, 8)) * 0.1).astype(np.float32)
    v = kernel(u, w)
    print("out", v.shape, v.dtype, np.abs(v).max())


# revision 24
# speedup vs baseline: 7916.1282x; 1.0077x over previous
"""CapsNet dynamic-routing layer on 8 Trainium2 NeuronCores (Bass/Tile).

reference math (per batch element b):
  u_hat[b,i,o,j] = sum_d W[i,o,j,d] * u[b,i,d]        (never materialized)
  bl = 0; for r in 0..2:
    c = softmax_o(bl); s[b,o,j] = sum_i c*u_hat; v = squash(s)
    if r < 2: bl += sum_j u_hat*v
  return v  [B, 10, 16]

Distribution: pure data parallel, batch 512 -> 64 per core x 8 cores;
weights replicated.  Per-core: b=64, i=1152=9*128, o=10, j=16, d=8.

Key layout trick: o is mapped to PE column/row strips as g=o%4 (strip)
and sl=o//4 (slot), consistently across the s-matmuls (col-tiled),
squash (strip-local), the agreement matmuls (row-tiled), and the
output transposes - so no partition-moving shuffles are ever needed.
The softmax splits o as h=o//5 across partition halves (paired with
the agreement-pass PSUM packing) and o5=o%5 along free.
"""
import sys

sys.path.insert(0, "/opt/trn_rl_repo")

import numpy as np
import ml_dtypes
from contextlib import ExitStack

from concourse import bacc, mybir, hw_specs
from concourse.tile import TileContext
from concourse.bass_utils import run_bass_kernel_spmd

BF16 = mybir.dt.bfloat16
F32 = mybir.dt.float32
AX = mybir.AxisListType
ALU = mybir.AluOpType
ACTF = mybir.ActivationFunctionType
bfnp = ml_dtypes.bfloat16

B = 64
I = 1152
T = 9
O = 10
J = 16
D = 8
EPS = 1e-06
N_CORES = 8
KFLAT = D * I          # 9216 (d-major flat)
NCH = KFLAT // 512     # 18

_cache = {}

# Route every activation through the one table set that has exp+ln+copy,
# so the ACT engine never reloads tables mid-kernel.  Entry order (and
# hence act_func_set_id indices) is preserved.
_KEEP_SET = "natural_log_exp_and_others"


def _patched_tables(arch):
    full = {k: set(v) for k, v in hw_specs.get_activation_tables(arch).items()}
    keep = full[_KEEP_SET]
    return {k: (v if k == _KEEP_SET else v - keep) for k, v in full.items()}


import os
if os.environ.get('ACT_PATCH', '1') == '1':
    bacc.get_activation_tables = _patched_tables


def build_nc():
    nc = bacc.Bacc()
    ws_d = nc.dram_tensor("ws", [128, T, D, O, J], BF16, kind="ExternalInput")
    wb_d = nc.dram_tensor("wb", [128, 3 * KFLAT], BF16, kind="ExternalInput")
    ui_d = nc.dram_tensor("ui", [128, T, D, B], BF16, kind="ExternalInput")
    ur_d = nc.dram_tensor("ur", [128, KFLAT], BF16, kind="ExternalInput")
    cid2_d = nc.dram_tensor("cid2", [128, 64], F32, kind="ExternalInput")
    cid16s_d = nc.dram_tensor("cid16s", [128, 16], F32, kind="ExternalInput")
    cones128_d = nc.dram_tensor("cones128", [128, 1], F32, kind="ExternalInput")
    cones1_d = nc.dram_tensor("cones1", [1, 16], F32, kind="ExternalInput")
    vout_d = nc.dram_tensor("vout", [B, O, J], F32, kind="ExternalOutput")

    with TileContext(nc) as tc, ExitStack() as ctx:
        static = ctx.enter_context(tc.tile_pool(name="static", bufs=1))
        work = ctx.enter_context(tc.tile_pool(name="work", bufs=1))
        cupool = ctx.enter_context(tc.tile_pool(name="cup", bufs=2))
        psA = ctx.enter_context(tc.tile_pool(name="psA", bufs=1, space="PSUM"))
        psB = ctx.enter_context(tc.tile_pool(name="psB", bufs=3, space="PSUM"))
        psC = ctx.enter_context(tc.tile_pool(name="psC", bufs=2, space="PSUM"))
        psD = ctx.enter_context(tc.tile_pool(name="psD", bufs=2, space="PSUM"))

        ws = static.tile([128, T, D, O, J], BF16, name="ws")
        wb = static.tile([128, 3 * KFLAT], BF16, name="wb")
        ui = static.tile([128, T, D, B], BF16, name="ui")
        ur = static.tile([128, KFLAT], BF16, name="ur")
        cid2 = static.tile([128, 64], F32, name="cid2")
        cid16s = static.tile([128, 16], F32, name="cid16s")
        cones128 = static.tile([128, 1], F32, name="cones128")
        cones1 = static.tile([1, 16], F32, name="cones1")
        nc.sync.dma_start(out=ws, in_=ws_d[:, :, :, :, :])
        nc.sync.dma_start(out=wb, in_=wb_d[:, :])
        nc.sync.dma_start(out=ui, in_=ui_d[:, :, :, :])
        nc.sync.dma_start(out=ur, in_=ur_d[:, :])
        nc.sync.dma_start(out=cid2, in_=cid2_d[:, :])
        nc.sync.dma_start(out=cid16s, in_=cid16s_d[:, :])
        nc.sync.dma_start(out=cones128, in_=cones128_d[:, :])
        nc.sync.dma_start(out=cones1, in_=cones1_d[:, :])

        bl = work.tile([128, 5, I], F32, name="bl")
        c_t = work.tile([128, T, O, B], BF16, name="c_t")
        zh = work.tile([128, I], F32, name="zh")
        rz = work.tile([128, I], F32, name="rz")
        scratch = work.tile([128, KFLAT], BF16, name="scratch")
        ug = scratch
        e = scratch[:, 0 : 5 * I].rearrange("p (o i) -> p o i", o=5)
        s_sb = work.tile([128, 3, B], F32, name="s_sb")
        s2 = work.tile([128, 3, B], F32, name="s2")
        v_sb = work.tile([128, 3, B], F32, name="v_sb")
        v_st = work.tile([128, 3, B], BF16, name="v_st")
        sq_sb = work.tile([1, 4, 3, B], F32, name="sq_sb")
        t1p = work.tile([1, 768], F32, name="t1p")
        t2p = work.tile([1, 768], F32, name="t2p")
        den = work.tile([1, 768], F32, name="den")
        rec = work.tile([1, 768], F32, name="rec")
        v_t = work.tile([64, O, J], F32, name="v_t")
        eps1 = work.tile([1, 1], F32, name="eps1")
        nc.vector.memset(eps1, EPS)
        nc.vector.memset(s_sb.rearrange("p s b -> p (s b)"), 0.0)
        nc.vector.memset(sq_sb.rearrange("p g s b -> p (g s b)"), 0.0)

        PAIRS = [(0, 1), (2, 3), (4, 5), (6, 7), (8, 9)]

        def m1_pair(pair, rhs_of, scale):
            """col-tiled s matmuls for an o-pair -> s_sb strips."""
            ps = psA.tile([128, B], F32, name="m1ps", tag="m1ps")
            for t in range(T):
                for d in range(D):
                    for o in pair:
                        g = o % 4
                        nc.tensor.matmul(
                            ps[32 * g : 32 * g + 16, :],
                            ws[:, t, d, o, :],
                            rhs_of(o)[:, t, d, :],
                            start=(t == 0 and d == 0),
                            stop=(t == T - 1 and d == D - 1),
                            tile_position=(0, 32 * g),
                        )
            for o in pair:
                g, slot = o % 4, o // 4
                nc.scalar.mul(s_sb[32 * g : 32 * g + 16, slot, :],
                              ps[32 * g : 32 * g + 16, :], scale)

        def squash():
            """v_sb = squash(s_sb) with j on partitions (strip-local)."""
            sf = s_sb.rearrange("p s b -> p (s b)")
            s2f = s2.rearrange("p s b -> p (s b)")
            nc.vector.tensor_tensor(s2f, sf, sf, op=ALU.mult)
            for g in range(4):
                nsl = 3 if g < 2 else 2
                sqg = psD.tile([1, 3 * B], F32, name="sqg", tag="sqps")
                nc.tensor.matmul(
                    sqg[:, 0 : nsl * B],
                    cones128[32 * g : 32 * g + 16, :],
                    s2[32 * g : 32 * g + 16, 0:nsl, :],
                    start=True, stop=True,
                    tile_position=(32 * g, 0),
                )
                # scatter group's o-slices (o = g + 4*sl) into sq_sb
                nc.vector.tensor_copy(
                    sq_sb[:, g, 0:nsl, :],
                    sqg[:, 0 : nsl * B].rearrange("p (s b) -> p s b", s=nsl),
                )
            # o-major view of sq_sb: o = g + 4*sl  ->  dims (sl, g, b)
            sqv = sq_sb.transpose([0, 2, 1, 3])
            def _v(ap):
                return ap.rearrange("p (s g b) -> p s g b", s=3, g=4)
            nc.scalar.activation(_v(t1p), sqv, ACTF.Ln, bias=eps1)
            nc.scalar.activation(t2p, t1p, ACTF.Exp, scale=0.5)
            nc.vector.tensor_scalar_add(_v(den), sqv, 1.0)
            nc.vector.tensor_tensor(den, den, t2p, op=ALU.mult)
            nc.vector.reciprocal_approx_accurate(rec, den, t1p)
            nc.vector.tensor_tensor(_v(den), sqv, _v(rec), op=ALU.mult)
            mrep = psC.tile([128, 3 * B], F32, name="mrep", tag="miscps")
            nc.vector.memset(mrep, 0.0)
            for o in range(O):
                g, sl = o % 4, o // 4
                nc.tensor.matmul(
                    mrep[32 * g : 32 * g + 16, 64 * sl : 64 * (sl + 1)],
                    cones1,
                    den[:, 64 * o : 64 * (o + 1)],
                    start=True, stop=True,
                    tile_position=(0, 32 * g),
                )
            vf = v_sb.rearrange("p s b -> p (s b)")
            nc.vector.tensor_tensor(vf, sf, mrep, op=ALU.mult)

        def m2_b2(it):
            """bl (+)= sum_j u_hat * v   (g = W.T@v row-tiled; u*g; d-tree)."""
            nc.vector.tensor_copy(v_st.rearrange("p s b -> p (s b)"),
                                  v_sb.rearrange("p s b -> p (s b)"))
            for p in range(5):
                for n in range(NCH // 2):
                    for nn in (n, n + 9):
                        ps = psB.tile([128, 512], F32, name="m2ps", tag="m2ps")
                        for half, o in ((0, p), (1, p + 5)):
                            g, sl = o % 4, o // 4
                            nc.tensor.matmul(
                                ps[64 * half : 64 * half + 64, :],
                                v_st[32 * g : 32 * g + 16, sl, :],
                                wb[32 * g : 32 * g + 16,
                                   sl * KFLAT + 512 * nn : sl * KFLAT + 512 * (nn + 1)],
                                start=True, stop=True,
                                tile_position=(32 * g, 64 * half),
                            )
                        nc.scalar.copy(ug[:, 512 * nn : 512 * (nn + 1)], ps)
                    for nn in (n, n + 9):
                        sl = slice(512 * nn, 512 * (nn + 1))
                        nc.vector.tensor_tensor(ug[:, sl], ug[:, sl], ur[:, sl],
                                                op=ALU.mult)
                    sl = slice(512 * (n + 9), 512 * (n + 10))
                    nc.vector.tensor_tensor(
                        ug[:, sl], ug[:, 512 * n : 512 * (n + 1)],
                        ug[:, sl], op=ALU.add)
                h, q = KFLAT // 2, KFLAT // 4
                # l1 lives in [h:KFLAT); fold its halves into [h:h+q)
                nc.vector.tensor_tensor(ug[:, h : h + q], ug[:, h : h + q],
                                        ug[:, h + q : KFLAT], op=ALU.add)
                l3 = ug[:, h + q : h + q + 2 * I].bitcast(F32)
                nc.vector.tensor_tensor(l3, ug[:, h : h + I],
                                        ug[:, h + I : h + 2 * I], op=ALU.add)
                if it == 0:
                    nc.vector.tensor_copy(bl[:, p, :], l3)
                else:
                    nc.vector.tensor_tensor(bl[:, p, :], bl[:, p, :], l3,
                                            op=ALU.add)

        def softmax():
            """e := c = softmax_o(bl); c -> c_t (i-partitioned) via DMA-T."""
            nc.scalar.activation(e[:, :, :], bl[:, :, :], ACTF.Exp)
            nc.vector.tensor_tensor(zh, e[:, 0, :], e[:, 1, :], op=ALU.add)
            nc.vector.tensor_tensor(rz, e[:, 2, :], e[:, 3, :], op=ALU.add)
            nc.vector.tensor_tensor(zh, zh, e[:, 4, :], op=ALU.add)
            nc.vector.tensor_tensor(zh, zh, rz, op=ALU.add)
            for n in range(3):
                sl = slice(384 * n, 384 * (n + 1))
                zp = psC.tile([128, 384], F32, name="zswap", tag="miscps")
                nc.tensor.matmul(zp[0:64, :], cid2[64:128, :], zh[64:128, sl],
                                 start=True, stop=True, tile_position=(64, 0))
                nc.tensor.matmul(zp[64:128, :], cid2[0:64, :], zh[0:64, sl],
                                 start=True, stop=True, tile_position=(0, 64))
                nc.vector.tensor_tensor(zh[:, sl], zh[:, sl], zp, op=ALU.add)
            nc.vector.reciprocal_approx_fast(rz, zh)
            for o5 in range(5):
                nc.vector.tensor_tensor(e[:, o5, :], e[:, o5, :], rz,
                                        op=ALU.mult)
            for o in range(O):
                o5, hh = o % 5, o // 5
                for t in range(T):
                    nc.sync.dma_start_transpose(
                        out=c_t[:, t, o, :],
                        in_=e[64 * hh : 64 * hh + 64, o5,
                              128 * t : 128 * (t + 1)],
                    )

        # ========================= flow =========================
        import os as _os
        STAGE = int(_os.environ.get("FLOW_STAGE", "99"))
        for it in range(3):
            if it > 0 and STAGE < 4:
                break
            if it == 0:
                for pair in PAIRS:
                    m1_pair(pair, lambda o: ui, 0.1)
            else:
                for pair in PAIRS:
                    cus = {}
                    for o in pair:
                        cu = cupool.tile([128, T, D, B], BF16, name="cu",
                                         tag="cu")
                        nc.vector.tensor_tensor(
                            cu[:, :, :, :],
                            c_t[:, :, o, :].unsqueeze(2).broadcast_to(
                                [128, T, D, B]),
                            ui[:, :, :, :],
                            op=ALU.mult,
                        )
                        cus[o] = cu
                    m1_pair(pair, lambda o: cus[o], 1.0)
            if STAGE >= 1:
                squash()
            if it < 2 and STAGE >= 2:
                m2_b2(it)
                if STAGE >= 3:
                    softmax()

        if STAGE < 1:
            nc.vector.memset(v_sb.rearrange("p s b -> p (s b)"), 0.5)
        for o in range(O):
            g, sl = o % 4, o // 4
            tp = psC.tile([64, J], F32, name="vtp", tag="miscps")
            nc.tensor.transpose(tp, v_sb[32 * g : 32 * g + 16, sl, :],
                                cid16s[32 * g : 32 * g + 16, :],
                                tile_position=(32 * g, 0))
            nc.scalar.copy(v_t[:, o, :], tp)
        nc.sync.dma_start(out=vout_d[:, :, :], in_=v_t)

    nc.finalize()
    return nc


def _host_prep(u, weights):
    """Per-core input maps. u [512,1152,8] f32, weights [1152,10,16,8] f32."""
    W = np.asarray(weights, dtype=np.float32)
    u = np.asarray(u, dtype=np.float32)
    ws = np.ascontiguousarray(
        W.reshape(T, 128, O, J, D).transpose(1, 0, 4, 2, 3)
    ).astype(bfnp)  # [128, T, D, O, J]
    wt = W.transpose(1, 2, 3, 0)  # [o, j, d, i]
    wb = np.zeros((128, 3 * KFLAT), dtype=bfnp)
    for o in range(O):
        g, sl = o % 4, o // 4
        wb[32 * g : 32 * g + 16, sl * KFLAT : (sl + 1) * KFLAT] = (
            wt[o].reshape(J, KFLAT).astype(bfnp)
        )
    cid2 = np.concatenate([np.eye(64, dtype=np.float32)] * 2, axis=0)
    cid16s = np.zeros((128, 16), dtype=np.float32)
    for g in range(4):
        cid16s[32 * g : 32 * g + 16, :] = np.eye(16, dtype=np.float32)
    cones128 = np.ones((128, 1), dtype=np.float32)
    cones1 = np.ones((1, 16), dtype=np.float32)

    base = {
        "ws": ws, "wb": wb, "cid2": cid2, "cid16s": cid16s,
        "cones128": cones128, "cones1": cones1,
    }
    in_maps = []
    for c in range(N_CORES):
        uc = u[c * B : (c + 1) * B]  # [64, 1152, 8]
        ui = np.ascontiguousarray(
            uc.reshape(B, T, 128, D).transpose(2, 1, 3, 0)
        ).astype(bfnp)  # [128, T, D, B]
        urh = np.ascontiguousarray(uc.transpose(0, 2, 1)).reshape(B, KFLAT)
        ur = np.concatenate([urh, urh], axis=0).astype(bfnp)  # [128, KFLAT]
        in_maps.append({**base, "ui": ui, "ur": ur})
    return in_maps


def kernel(u, weights):
    if "nc" not in _cache:
        _cache["nc"] = build_nc()
    nc = _cache["nc"]
    in_maps = _host_prep(u, weights)
    res = run_bass_kernel_spmd(nc, in_maps, core_ids=list(range(N_CORES)))
    out = np.concatenate([res.results[c]["vout"] for c in range(N_CORES)], axis=0)
    return out.astype(np.float32)


if __name__ == "__main__":
    rng = np.random.default_rng(0)
    u = rng.standard_normal((512, 1152, 8), dtype=np.float32)
    w = (rng.standard_normal((1152, 10, 16, 8)) * 0.1).astype(np.float32)
    v = kernel(u, w)
    print("out", v.shape, v.dtype, np.abs(v).max())
